# revision 12
# baseline (speedup 1.0000x reference)
# Dense-MoE (all experts active) Trainium2 kernel, expert-parallel over 8
# NeuronCores. Each core computes its expert's 2-layer MLP over all tokens:
#   fe_e = gelu(h @ W1[e] + b1[e]) @ (probs[e] * W2[e])
# then a chunked fp16 ReduceScatter(add) across the 8 cores sums the expert
# contributions; the host reassembles the full [B, D] output and adds the
# (token-independent) bias term sum_e probs[e]*b2[e].
#
# Layout: activations stay transposed on-chip; L2 output is [D, tokens].
#   hT   [IN, B]    fp16, pre-transposed on the host, fully SBUF-resident
#   hidT [H, blk]   = (W1 block).T @ hT per 128-row chunk, gelu+b1 via ACT
#   feT  [D, blk]   = (W2 block).T @ hidT accumulated over H chunks
#
# Structure notes (from HW trace analysis):
# - Under full 8-core load the PE pair period is ~263ns per 512-col fp16
#   matmul (chip-level clock throttle; a single core runs the same stream at
#   216ns). 2048 matmuls -> ~537us is the compute floor; everything else
#   here is about not adding to it.
# - W1 is repacked host-side so each L1 m-pass needs one contiguous 0.25MB
#   DMA, and hT is packed so each token block is one contiguous DMA (each
#   dma_start costs ~650ns of sequencer issue time). The first block's hT
#   arrives in 0.5MB quarters so the first chains start ~10us in.
# - PSUM bank groups rotate through all 8 banks so the Gelu/drain engines
#   never gate the next pass's matmuls.
# - ReduceScatter wall time is ~10us fixed + payload/(~60GB/s) on the one
#   CC core, so blocks shrink over the run (1024,1024,1536,512 tokens):
#   early blocks ship big chunks during ample compute windows; the final
#   512-token block ships four 0.25MB Dc-pair chunks so the last RS is
#   short and hits an idle CC queue. fe drains ride the scalar queue and
#   out writes the gpsimd queue to avoid head-of-line blocking on loads.
import os
import sys

sys.path.insert(0, "/opt/trn_rl_repo")

import numpy as np

import concourse.mybir as mybir
from concourse import bacc, tile

B, E, IN, H, D = 4096, 8, 1024, 2048, 1024
NCORES = 8
P = 128
KC1 = IN // P             # 8 contraction chunks, layer 1
MC1 = H // P              # 16 H chunks (layer-1 output rows)
DC2 = D // P              # 8 D chunks (layer-2 output rows)

BLOCKS = [1024, 1024, 1536, 512]          # tokens per phase-block
TOFF = [sum(BLOCKS[:i]) for i in range(len(BLOCKS) + 1)]
NBLK = len(BLOCKS)
MAXBLK = max(BLOCKS)

# ReduceScatter chunk schedule: (block, dc_lo, n_dc).
CHUNKS = [
    (0, 0, 8),
    (1, 0, 8),
    (2, 0, 4), (2, 4, 4),
    (3, 0, 2), (3, 2, 2), (3, 4, 2), (3, 6, 2),
]

F32 = mybir.dt.float32

_CACHE = {}


def build(mm_dtype_name="float16", rs_dtype_name="float16"):
    mm_dt = getattr(mybir.dt, mm_dtype_name)
    rs_dt = getattr(mybir.dt, rs_dtype_name)
    assert mybir.dt.size(mm_dt) == 2, "matmul path requires a 16-bit dtype"
    nc = bacc.Bacc("TRN2", target_bir_lowering=False)

    # htp: per-block contiguous segments; within block b (BLK tokens):
    # htp[p, TOFF[b]*KC1 + k*BLK + t] = h[TOFF[b] + t, k*P + p]
    htp = nc.declare_dram_parameter("htp", [P, KC1 * B], mm_dt,
                                    isOutput=False)
    # w1m[m*P + p, k*P + c] = W1[k*P + p, m*P + c]
    w1m = nc.declare_dram_parameter("w1m", [MC1 * P, IN], mm_dt,
                                    isOutput=False)
    b1t = nc.declare_dram_parameter("b1t", [P, MC1], F32, isOutput=False)
    w2 = nc.declare_dram_parameter("w2", [H, D], mm_dt, isOutput=False)
    # one output param per distinct chunk column count
    out_rows = {}
    for (b, dc_lo, n) in CHUNKS:
        cols = BLOCKS[b]
        out_rows[cols] = out_rows.get(cols, 0) + n * P // NCORES
    outs = {
        cols: nc.declare_dram_parameter(f"out{cols}", [rows, cols], rs_dt,
                                        isOutput=True)
        for cols, rows in out_rows.items()
    }
    out_off = {}
    _pos = {cols: 0 for cols in out_rows}
    for (b, dc_lo, n) in CHUNKS:
        cols = BLOCKS[b]
        out_off[(b, dc_lo)] = _pos[cols]
        _pos[cols] += n * P // NCORES

    with tile.TileContext(nc) as tc:
        with (
            tc.tile_pool(name="weights", bufs=1) as wpool,
            tc.tile_pool(name="consts", bufs=1) as cpool,
            tc.tile_pool(name="ht", bufs=1) as ht_pool,
            tc.tile_pool(name="hid", bufs=MC1) as hid_pool,
            tc.tile_pool(name="fe", bufs=2) as fe_pool,
            tc.tile_pool(name="ps", bufs=8, space="PSUM") as ps_pool,
            tc.tile_pool(name="dram", bufs=2, space="DRAM") as dram_pool,
        ):
            # --- input DMAs, ordered for the earliest possible L1 start ---
            ht_tiles = {}
            h0q = []
            w1_first = []
            for q in range(4):  # block0 in 2-slab quarters
                t_ = ht_pool.tile([P, 2 * BLOCKS[0]], mm_dt, tag=f"ht0_{q}")
                nc.sync.dma_start(
                    t_[:],
                    htp[:, q * 2 * BLOCKS[0]:(q + 1) * 2 * BLOCKS[0]],
                )
                h0q.append(t_)
                if q == 0:  # the first m-pass's weights ride 2nd in line
                    t_ = wpool.tile([P, IN], mm_dt, tag="w1_0")
                    nc.sync.dma_start(t_[:], w1m[0:P, :])
                    w1_first.append(t_)

            def ht_slab(b, k):
                if b == 0:
                    return h0q[k // 2][:, (k % 2) * BLOCKS[0]:
                                      (k % 2 + 1) * BLOCKS[0]]
                t_ = ht_tiles[b]
                return t_[:, k * BLOCKS[b]:(k + 1) * BLOCKS[b]]

            w1_sb = list(w1_first)
            for m in range(1, MC1):
                t_ = wpool.tile([P, IN], mm_dt, tag=f"w1_{m}")
                nc.sync.dma_start(t_[:], w1m[m * P:(m + 1) * P, :])
                w1_sb.append(t_)
                if m == 1:
                    b1_sb = cpool.tile([P, MC1], F32, tag="b1")
                    nc.sync.dma_start(b1_sb[:], b1t[:])
                    t2 = ht_pool.tile([P, KC1 * BLOCKS[1]], mm_dt, tag="ht_1")
                    nc.sync.dma_start(
                        t2[:],
                        htp[:, TOFF[1] * KC1:TOFF[2] * KC1],
                    )
                    ht_tiles[1] = t2
            w2_sb = []
            for hc in range(MC1):
                t_ = wpool.tile([P, D], mm_dt, tag=f"w2_{hc}")
                nc.sync.dma_start(t_[:], w2[hc * P:(hc + 1) * P, :])
                w2_sb.append(t_)
            for b in range(2, NBLK):
                t_ = ht_pool.tile([P, KC1 * BLOCKS[b]], mm_dt, tag=f"ht_{b}")
                nc.sync.dma_start(
                    t_[:], htp[:, TOFF[b] * KC1:TOFF[b + 1] * KC1]
                )
                ht_tiles[b] = t_

            for b in range(NBLK):
                blk = BLOCKS[b]
                nbank = blk // 512
                # --- L1: hidT[m] = gelu((W1 block m).T @ hT + b1[m]) ---
                hid_sb = []
                for m in range(MC1):
                    banks = [
                        ps_pool.tile([P, 512], F32, tag="ps", name=f"ps{j}")
                        for j in range(nbank)
                    ]
                    for k in range(KC1):
                        for j in range(nbank):
                            nc.tensor.matmul(
                                banks[j][:],
                                w1_sb[m][:, k * P:(k + 1) * P],
                                ht_slab(b, k)[:, j * 512:(j + 1) * 512],
                                start=(k == 0),
                                stop=(k == KC1 - 1),
                            )
                    hm = hid_pool.tile([P, MAXBLK], mm_dt, tag="hid")
                    for j in range(nbank):
                        nc.scalar.activation(
                            hm[:, j * 512:(j + 1) * 512],
                            banks[j][:],
                            mybir.ActivationFunctionType.Gelu,
                            bias=b1_sb[:, m:m + 1],
                            scale=1.0,
                        )
                    hid_sb.append(hm)

                # --- L2 + chunked ReduceScatter per the schedule ---
                chunks = [c for c in CHUNKS if c[0] == b]
                ci = 0
                fe_chunk = None
                for dc in range(DC2):
                    blk_, dc_lo, dc_n = chunks[ci]
                    if dc == dc_lo:
                        fe_chunk = dram_pool.tile(
                            [dc_n * P, blk], rs_dt, tag=f"fe{dc_n}_{blk}",
                            name="fe_chunk", bufs=2,
                        )
                    banks = [
                        ps_pool.tile([P, 512], F32, tag="ps", name=f"ps{j}")
                        for j in range(nbank)
                    ]
                    for hc in range(MC1):
                        for j in range(nbank):
                            nc.tensor.matmul(
                                banks[j][:],
                                w2_sb[hc][:, dc * P:(dc + 1) * P],
                                hid_sb[hc][:, j * 512:(j + 1) * 512],
                                start=(hc == 0),
                                stop=(hc == MC1 - 1),
                            )
                    fe_sb = fe_pool.tile([P, MAXBLK], rs_dt, tag="fe")
                    for j in range(nbank):
                        nc.scalar.activation(
                            fe_sb[:, j * 512:(j + 1) * 512],
                            banks[j][:],
                            mybir.ActivationFunctionType.Copy,
                        )
                    nc.scalar.dma_start(
                        fe_chunk[(dc - dc_lo) * P:(dc - dc_lo + 1) * P, :],
                        fe_sb[:, 0:blk],
                    )
                    if dc == dc_lo + dc_n - 1:
                        rs_chunk = dram_pool.tile(
                            [dc_n * P // NCORES, blk], rs_dt,
                            tag=f"rs{dc_n}_{blk}", name="rs_chunk", bufs=2,
                        )
                        nc.gpsimd.collective_compute(
                            "ReduceScatter",
                            mybir.AluOpType.add,
                            replica_groups=[list(range(NCORES))],
                            ins=[fe_chunk[:]],
                            outs=[rs_chunk[:]],
                        )
                        off = out_off[(b, dc_lo)]
                        nc.sync.dma_start(
                            outs[blk][off:off + dc_n * P // NCORES, :],
                            rs_chunk[:],
                        )
                        ci += 1

    nc.finalize()
    return nc


def _get_nc(mm_dtype_name, rs_dtype_name):
    key = (mm_dtype_name, rs_dtype_name)
    if key not in _CACHE:
        _CACHE[key] = build(mm_dtype_name, rs_dtype_name)
    return _CACHE[key]


def _run(inputs, mm_dtype_name="float16", trace=False):
    from concourse.bass_utils import run_bass_kernel_spmd

    import ml_dtypes

    rs_dtype_name = os.environ.get("MOE_RS_DTYPE", "float16")
    np_mm = {"bfloat16": ml_dtypes.bfloat16, "float16": np.float16}[
        mm_dtype_name
    ]
    h = np.ascontiguousarray(np.asarray(inputs["h"], dtype=np.float32))
    hT = h.T.astype(np_mm)  # [IN, B]
    htp = np.concatenate(
        [
            np.ascontiguousarray(
                hT[:, TOFF[b]:TOFF[b + 1]]
                .reshape(KC1, P, BLOCKS[b]).transpose(1, 0, 2)
                .reshape(P, KC1 * BLOCKS[b])
            )
            for b in range(NBLK)
        ],
        axis=1,
    )
    gate_logits = np.asarray(inputs["gate_logits"], dtype=np.float64)
    W1 = np.asarray(inputs["W1"], dtype=np.float32)
    b1 = np.asarray(inputs["b1"], dtype=np.float32)
    W2 = np.asarray(inputs["W2"], dtype=np.float32)
    b2 = np.asarray(inputs["b2"], dtype=np.float32)

    # gate: softmax over E (uniform for zero logits); fold into W2 per expert
    z = np.exp(gate_logits - gate_logits.max())
    probs = (z / z.sum()).astype(np.float32)

    in_maps = []
    for e in range(NCORES):
        w1e = W1[e].astype(np_mm)                      # [IN, H]
        w1m = np.ascontiguousarray(
            w1e.reshape(KC1, P, MC1, P).transpose(2, 1, 0, 3)
            .reshape(MC1 * P, IN)
        )
        in_maps.append({
            "htp": htp,
            "w1m": w1m,
            "b1t": np.ascontiguousarray(b1[e].reshape(MC1, P).T),  # [P,MC1]
            "w2": np.ascontiguousarray((W2[e] * probs[e]).astype(np_mm)),
        })

    nc = _get_nc(mm_dtype_name, rs_dtype_name)
    res = run_bass_kernel_spmd(nc, in_maps, list(range(NCORES)), trace=trace)

    # Reassemble. Chunk (b, dc_lo, n): core r's rows in out<cols> are global
    # D rows dc_lo*128 + r*(n*16) + i, tokens TOFF[b]..TOFF[b+1].
    out_pos = {}
    feT = np.empty((D, B), dtype=np.float32)
    for r in range(NCORES):
        o_by_cols = {}
        pos = {}
        for (b, dc_lo, n) in CHUNKS:
            cols = BLOCKS[b]
            if cols not in o_by_cols:
                o_by_cols[cols] = np.asarray(
                    res.results[r][f"out{cols}"], dtype=np.float32
                )
                pos[cols] = 0
            rpc = n * P // NCORES
            o = o_by_cols[cols]
            feT[dc_lo * P + r * rpc: dc_lo * P + (r + 1) * rpc,
                TOFF[b]:TOFF[b + 1]] = o[pos[cols]:pos[cols] + rpc, :]
            pos[cols] += rpc
    final = feT.T.copy()
    final += (probs @ b2)[None, :]  # token-independent bias term
    return final, res


def kernel(**inputs):
    mm_dtype_name = os.environ.get("MOE_MM_DTYPE", "float16")
    final, _ = _run(inputs, mm_dtype_name=mm_dtype_name, trace=False)
    return final


# revision 13
# speedup vs baseline: 1.0206x; 1.0206x over previous
# Dense-MoE (all experts active) Trainium2 kernel, expert-parallel over 8
# NeuronCores. Each core computes its expert's 2-layer MLP over all tokens:
#   fe_e = gelu(h @ W1[e] + b1[e]) @ (probs[e] * W2[e])
# then a chunked fp16 ReduceScatter(add) across the 8 cores sums the expert
# contributions; the host reassembles the full [B, D] output and adds the
# (token-independent) bias term sum_e probs[e]*b2[e].
#
# Layout: activations stay transposed on-chip; L2 output is [D, tokens].
#   hT   [IN, B]    fp16, pre-transposed on the host, fully SBUF-resident
#   hidT [H, blk]   = (W1 block).T @ hT per 128-row chunk, gelu+b1 via ACT
#   feT  [D, blk]   = (W2 block).T @ hidT accumulated over H chunks
#
# Structure notes (from HW trace analysis):
# - Under full 8-core load the PE pair period is ~263ns per 512-col fp16
#   matmul (chip-level clock throttle; a single core runs the same stream at
#   216ns). 2048 matmuls -> ~537us is the compute floor; everything else
#   here is about not adding to it.
# - W1 is repacked host-side so each L1 m-pass needs one contiguous 0.25MB
#   DMA, and hT is packed so each token block is one contiguous DMA (each
#   dma_start costs ~650ns of sequencer issue time). The first block's hT
#   arrives in 0.5MB quarters so the first chains start ~10us in.
# - PSUM bank groups rotate through all 8 banks so the Gelu/drain engines
#   never gate the next pass's matmuls.
# - ReduceScatter wall time is ~10us fixed + payload/(~60GB/s) on the one
#   CC core, so blocks shrink over the run (1024,1024,1536,512 tokens):
#   early blocks ship big chunks during ample compute windows; the final
#   512-token block ships four 0.25MB Dc-pair chunks so the last RS is
#   short and hits an idle CC queue. fe drains ride the scalar queue and
#   out writes the gpsimd queue to avoid head-of-line blocking on loads.
import os
import sys

sys.path.insert(0, "/opt/trn_rl_repo")

import numpy as np

import concourse.mybir as mybir
from concourse import bacc, tile

B, E, IN, H, D = 4096, 8, 1024, 2048, 1024
NCORES = 8
P = 128
KC1 = IN // P             # 8 contraction chunks, layer 1
MC1 = H // P              # 16 H chunks (layer-1 output rows)
DC2 = D // P              # 8 D chunks (layer-2 output rows)

BLOCKS = [1024, 1024, 1536, 512]          # tokens per phase-block
TOFF = [sum(BLOCKS[:i]) for i in range(len(BLOCKS) + 1)]
NBLK = len(BLOCKS)
MAXBLK = max(BLOCKS)

# ReduceScatter chunk schedule: (block, dc_lo, n_dc).
CHUNKS = [
    (0, 0, 8),
    (1, 0, 8),
    (2, 0, 4), (2, 4, 4),
    (3, 0, 2), (3, 2, 2), (3, 4, 2), (3, 6, 2),
]

F32 = mybir.dt.float32

_CACHE = {}


def build(mm_dtype_name="float16", rs_dtype_name="float16"):
    mm_dt = getattr(mybir.dt, mm_dtype_name)
    rs_dt = getattr(mybir.dt, rs_dtype_name)
    assert mybir.dt.size(mm_dt) == 2, "matmul path requires a 16-bit dtype"
    nc = bacc.Bacc("TRN2", target_bir_lowering=False)

    # htp: per-block contiguous segments; within block b (BLK tokens):
    # htp[p, TOFF[b]*KC1 + k*BLK + t] = h[TOFF[b] + t, k*P + p]
    htp = nc.declare_dram_parameter("htp", [P, KC1 * B], mm_dt,
                                    isOutput=False)
    # w1m[m*P + p, k*P + c] = W1[k*P + p, m*P + c]
    w1m = nc.declare_dram_parameter("w1m", [MC1 * P, IN], mm_dt,
                                    isOutput=False)
    b1t = nc.declare_dram_parameter("b1t", [P, MC1], F32, isOutput=False)
    w2 = nc.declare_dram_parameter("w2", [H, D], mm_dt, isOutput=False)
    # one output param per distinct chunk column count
    out_rows = {}
    for (b, dc_lo, n) in CHUNKS:
        cols = BLOCKS[b]
        out_rows[cols] = out_rows.get(cols, 0) + n * P // NCORES
    outs = {
        cols: nc.declare_dram_parameter(f"out{cols}", [rows, cols], rs_dt,
                                        isOutput=True)
        for cols, rows in out_rows.items()
    }
    out_off = {}
    _pos = {cols: 0 for cols in out_rows}
    for (b, dc_lo, n) in CHUNKS:
        cols = BLOCKS[b]
        out_off[(b, dc_lo)] = _pos[cols]
        _pos[cols] += n * P // NCORES

    with tile.TileContext(nc) as tc:
        with (
            tc.tile_pool(name="weights", bufs=1) as wpool,
            tc.tile_pool(name="consts", bufs=1) as cpool,
            tc.tile_pool(name="ht", bufs=1) as ht_pool,
            tc.tile_pool(name="hid", bufs=MC1) as hid_pool,
            tc.tile_pool(name="fe", bufs=2) as fe_pool,
            tc.tile_pool(name="ps", bufs=8, space="PSUM") as ps_pool,
            tc.tile_pool(name="dram", bufs=2, space="DRAM") as dram_pool,
        ):
            # --- input DMAs, ordered for the earliest possible L1 start ---
            ht_tiles = {}
            h0q = []
            w1_first = []
            for q in range(4):  # block0 in 2-slab quarters
                t_ = ht_pool.tile([P, 2 * BLOCKS[0]], mm_dt, tag=f"ht0_{q}")
                nc.sync.dma_start(
                    t_[:],
                    htp[:, q * 2 * BLOCKS[0]:(q + 1) * 2 * BLOCKS[0]],
                )
                h0q.append(t_)
                if q == 0:  # the first m-pass's weights ride 2nd in line
                    t_ = wpool.tile([P, IN], mm_dt, tag="w1_0")
                    nc.sync.dma_start(t_[:], w1m[0:P, :])
                    w1_first.append(t_)

            def ht_slab(b, k):
                if b == 0:
                    return h0q[k // 2][:, (k % 2) * BLOCKS[0]:
                                      (k % 2 + 1) * BLOCKS[0]]
                t_ = ht_tiles[b]
                return t_[:, k * BLOCKS[b]:(k + 1) * BLOCKS[b]]

            w1_sb = list(w1_first)
            for m in range(1, MC1):
                t_ = wpool.tile([P, IN], mm_dt, tag=f"w1_{m}")
                nc.sync.dma_start(t_[:], w1m[m * P:(m + 1) * P, :])
                w1_sb.append(t_)
                if m == 1:  # first gelu needs the bias ~10us after mm0
                    b1_sb = cpool.tile([P, MC1], F32, tag="b1")
                    nc.sync.dma_start(b1_sb[:], b1t[:])
            t2 = ht_pool.tile([P, KC1 * BLOCKS[1]], mm_dt, tag="ht_1")
            nc.sync.dma_start(t2[:], htp[:, TOFF[1] * KC1:TOFF[2] * KC1])
            ht_tiles[1] = t2
            w2_sb = []
            for hc in range(MC1):
                t_ = wpool.tile([P, D], mm_dt, tag=f"w2_{hc}")
                nc.sync.dma_start(t_[:], w2[hc * P:(hc + 1) * P, :])
                w2_sb.append(t_)
            for b in range(2, NBLK):
                t_ = ht_pool.tile([P, KC1 * BLOCKS[b]], mm_dt, tag=f"ht_{b}")
                nc.sync.dma_start(
                    t_[:], htp[:, TOFF[b] * KC1:TOFF[b + 1] * KC1]
                )
                ht_tiles[b] = t_

            for b in range(NBLK):
                blk = BLOCKS[b]
                nbank = blk // 512
                # --- L1: hidT[m] = gelu((W1 block m).T @ hT + b1[m]) ---
                hid_sb = []
                for m in range(MC1):
                    banks = [
                        ps_pool.tile([P, 512], F32, tag="ps", name=f"ps{j}")
                        for j in range(nbank)
                    ]
                    for k in range(KC1):
                        for j in range(nbank):
                            nc.tensor.matmul(
                                banks[j][:],
                                w1_sb[m][:, k * P:(k + 1) * P],
                                ht_slab(b, k)[:, j * 512:(j + 1) * 512],
                                start=(k == 0),
                                stop=(k == KC1 - 1),
                            )
                    hm = hid_pool.tile([P, MAXBLK], mm_dt, tag="hid")
                    for j in range(nbank):
                        nc.scalar.activation(
                            hm[:, j * 512:(j + 1) * 512],
                            banks[j][:],
                            mybir.ActivationFunctionType.Gelu,
                            bias=b1_sb[:, m:m + 1],
                            scale=1.0,
                        )
                    hid_sb.append(hm)

                # --- L2 + chunked ReduceScatter per the schedule ---
                chunks = [c for c in CHUNKS if c[0] == b]
                ci = 0
                fe_chunk = None
                for dc in range(DC2):
                    blk_, dc_lo, dc_n = chunks[ci]
                    if dc == dc_lo:
                        fe_chunk = dram_pool.tile(
                            [dc_n * P, blk], rs_dt, tag=f"fe{dc_n}_{blk}",
                            name="fe_chunk", bufs=2,
                        )
                    banks = [
                        ps_pool.tile([P, 512], F32, tag="ps", name=f"ps{j}")
                        for j in range(nbank)
                    ]
                    for hc in range(MC1):
                        for j in range(nbank):
                            nc.tensor.matmul(
                                banks[j][:],
                                w2_sb[hc][:, dc * P:(dc + 1) * P],
                                hid_sb[hc][:, j * 512:(j + 1) * 512],
                                start=(hc == 0),
                                stop=(hc == MC1 - 1),
                            )
                    fe_sb = fe_pool.tile([P, MAXBLK], rs_dt, tag="fe")
                    for j in range(nbank):
                        nc.scalar.activation(
                            fe_sb[:, j * 512:(j + 1) * 512],
                            banks[j][:],
                            mybir.ActivationFunctionType.Copy,
                        )
                    nc.scalar.dma_start(
                        fe_chunk[(dc - dc_lo) * P:(dc - dc_lo + 1) * P, :],
                        fe_sb[:, 0:blk],
                    )
                    if dc == dc_lo + dc_n - 1:
                        rs_chunk = dram_pool.tile(
                            [dc_n * P // NCORES, blk], rs_dt,
                            tag=f"rs{dc_n}_{blk}", name="rs_chunk", bufs=2,
                        )
                        nc.gpsimd.collective_compute(
                            "ReduceScatter",
                            mybir.AluOpType.add,
                            replica_groups=[list(range(NCORES))],
                            ins=[fe_chunk[:]],
                            outs=[rs_chunk[:]],
                        )
                        off = out_off[(b, dc_lo)]
                        nc.sync.dma_start(
                            outs[blk][off:off + dc_n * P // NCORES, :],
                            rs_chunk[:],
                        )
                        ci += 1

    nc.finalize()
    return nc


def _get_nc(mm_dtype_name, rs_dtype_name):
    key = (mm_dtype_name, rs_dtype_name)
    if key not in _CACHE:
        _CACHE[key] = build(mm_dtype_name, rs_dtype_name)
    return _CACHE[key]


def _run(inputs, mm_dtype_name="float16", trace=False):
    from concourse.bass_utils import run_bass_kernel_spmd

    import ml_dtypes

    rs_dtype_name = os.environ.get("MOE_RS_DTYPE", "float16")
    np_mm = {"bfloat16": ml_dtypes.bfloat16, "float16": np.float16}[
        mm_dtype_name
    ]
    h = np.ascontiguousarray(np.asarray(inputs["h"], dtype=np.float32))
    hT = h.T.astype(np_mm)  # [IN, B]
    htp = np.concatenate(
        [
            np.ascontiguousarray(
                hT[:, TOFF[b]:TOFF[b + 1]]
                .reshape(KC1, P, BLOCKS[b]).transpose(1, 0, 2)
                .reshape(P, KC1 * BLOCKS[b])
            )
            for b in range(NBLK)
        ],
        axis=1,
    )
    gate_logits = np.asarray(inputs["gate_logits"], dtype=np.float64)
    W1 = np.asarray(inputs["W1"], dtype=np.float32)
    b1 = np.asarray(inputs["b1"], dtype=np.float32)
    W2 = np.asarray(inputs["W2"], dtype=np.float32)
    b2 = np.asarray(inputs["b2"], dtype=np.float32)

    # gate: softmax over E (uniform for zero logits); fold into W2 per expert
    z = np.exp(gate_logits - gate_logits.max())
    probs = (z / z.sum()).astype(np.float32)

    in_maps = []
    for e in range(NCORES):
        w1e = W1[e].astype(np_mm)                      # [IN, H]
        w1m = np.ascontiguousarray(
            w1e.reshape(KC1, P, MC1, P).transpose(2, 1, 0, 3)
            .reshape(MC1 * P, IN)
        )
        in_maps.append({
            "htp": htp,
            "w1m": w1m,
            "b1t": np.ascontiguousarray(b1[e].reshape(MC1, P).T),  # [P,MC1]
            "w2": np.ascontiguousarray((W2[e] * probs[e]).astype(np_mm)),
        })

    nc = _get_nc(mm_dtype_name, rs_dtype_name)
    res = run_bass_kernel_spmd(nc, in_maps, list(range(NCORES)), trace=trace)

    # Reassemble. Chunk (b, dc_lo, n): core r's rows in out<cols> are global
    # D rows dc_lo*128 + r*(n*16) + i, tokens TOFF[b]..TOFF[b+1].
    out_pos = {}
    feT = np.empty((D, B), dtype=np.float32)
    for r in range(NCORES):
        o_by_cols = {}
        pos = {}
        for (b, dc_lo, n) in CHUNKS:
            cols = BLOCKS[b]
            if cols not in o_by_cols:
                o_by_cols[cols] = np.asarray(
                    res.results[r][f"out{cols}"], dtype=np.float32
                )
                pos[cols] = 0
            rpc = n * P // NCORES
            o = o_by_cols[cols]
            feT[dc_lo * P + r * rpc: dc_lo * P + (r + 1) * rpc,
                TOFF[b]:TOFF[b + 1]] = o[pos[cols]:pos[cols] + rpc, :]
            pos[cols] += rpc
    final = feT.T.copy()
    final += (probs @ b2)[None, :]  # token-independent bias term
    return final, res


def kernel(**inputs):
    mm_dtype_name = os.environ.get("MOE_MM_DTYPE", "float16")
    final, _ = _run(inputs, mm_dtype_name=mm_dtype_name, trace=False)
    return final


# revision 14
# speedup vs baseline: 1.0384x; 1.0175x over previous
# Dense-MoE (all experts active) Trainium2 kernel, expert-parallel over 8
# NeuronCores. Each core computes its expert's 2-layer MLP over all tokens:
#   fe_e = gelu(h @ W1[e] + b1[e]) @ (probs[e] * W2[e])
# then a chunked fp16 ReduceScatter(add) across the 8 cores sums the expert
# contributions; the host reassembles the full [B, D] output and adds the
# (token-independent) bias term sum_e probs[e]*b2[e].
#
# Layout: activations stay transposed on-chip; L2 output is [D, tokens].
#   hT   [IN, B]    fp16, pre-transposed on the host, fully SBUF-resident
#   hidT [H, blk]   = (W1 block).T @ hT per 128-row chunk, gelu+b1 via ACT
#   feT  [D, blk]   = (W2 block).T @ hidT accumulated over H chunks
#
# Structure notes (from HW trace analysis):
# - Under full 8-core load the PE pair period is ~263ns per 512-col fp16
#   matmul (chip-level clock throttle; a single core runs the same stream at
#   216ns). 2048 matmuls -> ~537us is the compute floor; everything else
#   here is about not adding to it.
# - W1 is repacked host-side so each L1 m-pass needs one contiguous 0.25MB
#   DMA, and hT is packed so each token block is one contiguous DMA (each
#   dma_start costs ~650ns of sequencer issue time). The first block's hT
#   arrives in 0.5MB quarters so the first chains start ~10us in.
# - PSUM bank groups rotate through all 8 banks so the Gelu/drain engines
#   never gate the next pass's matmuls.
# - ReduceScatter wall time is ~10us fixed + payload/(~60GB/s) on the one
#   CC core, so blocks shrink over the run (1024,1024,1536,512 tokens):
#   early blocks ship big chunks during ample compute windows; the final
#   512-token block ships four 0.25MB Dc-pair chunks so the last RS is
#   short and hits an idle CC queue. fe drains ride the scalar queue and
#   out writes the gpsimd queue to avoid head-of-line blocking on loads.
import os
import sys

sys.path.insert(0, "/opt/trn_rl_repo")

import numpy as np

import concourse.mybir as mybir
from concourse import bacc, tile

B, E, IN, H, D = 4096, 8, 1024, 2048, 1024
NCORES = 8
P = 128
KC1 = IN // P             # 8 contraction chunks, layer 1
MC1 = H // P              # 16 H chunks (layer-1 output rows)
DC2 = D // P              # 8 D chunks (layer-2 output rows)

BLOCKS = [1024, 1024, 1536, 512]          # tokens per phase-block
TOFF = [sum(BLOCKS[:i]) for i in range(len(BLOCKS) + 1)]
NBLK = len(BLOCKS)
MAXBLK = max(BLOCKS)

# ReduceScatter chunk schedule: (block, dc_lo, n_dc).
CHUNKS = [
    (0, 0, 8),
    (1, 0, 8),
    (2, 0, 4), (2, 4, 4),
    (3, 0, 4), (3, 4, 4),
]

F32 = mybir.dt.float32

_CACHE = {}


def build(mm_dtype_name="float16", rs_dtype_name="float16"):
    mm_dt = getattr(mybir.dt, mm_dtype_name)
    rs_dt = getattr(mybir.dt, rs_dtype_name)
    assert mybir.dt.size(mm_dt) == 2, "matmul path requires a 16-bit dtype"
    nc = bacc.Bacc("TRN2", target_bir_lowering=False)

    # htp: per-block contiguous segments; within block b (BLK tokens):
    # htp[p, TOFF[b]*KC1 + k*BLK + t] = h[TOFF[b] + t, k*P + p]
    htp = nc.declare_dram_parameter("htp", [P, KC1 * B], mm_dt,
                                    isOutput=False)
    # w1m[m*P + p, k*P + c] = W1[k*P + p, m*P + c]
    w1m = nc.declare_dram_parameter("w1m", [MC1 * P, IN], mm_dt,
                                    isOutput=False)
    b1t = nc.declare_dram_parameter("b1t", [P, MC1], F32, isOutput=False)
    w2 = nc.declare_dram_parameter("w2", [H, D], mm_dt, isOutput=False)
    # one output param per distinct chunk column count
    out_rows = {}
    for (b, dc_lo, n) in CHUNKS:
        cols = BLOCKS[b]
        out_rows[cols] = out_rows.get(cols, 0) + n * P // NCORES
    outs = {
        cols: nc.declare_dram_parameter(f"out{cols}", [rows, cols], rs_dt,
                                        isOutput=True)
        for cols, rows in out_rows.items()
    }
    out_off = {}
    _pos = {cols: 0 for cols in out_rows}
    for (b, dc_lo, n) in CHUNKS:
        cols = BLOCKS[b]
        out_off[(b, dc_lo)] = _pos[cols]
        _pos[cols] += n * P // NCORES

    with tile.TileContext(nc) as tc:
        with (
            tc.tile_pool(name="weights", bufs=1) as wpool,
            tc.tile_pool(name="consts", bufs=1) as cpool,
            tc.tile_pool(name="ht", bufs=1) as ht_pool,
            tc.tile_pool(name="hid", bufs=MC1) as hid_pool,
            tc.tile_pool(name="fe", bufs=2) as fe_pool,
            tc.tile_pool(name="ps", bufs=8, space="PSUM") as ps_pool,
            tc.tile_pool(name="dram", bufs=2, space="DRAM") as dram_pool,
        ):
            # --- input DMAs, ordered for the earliest possible L1 start ---
            ht_tiles = {}
            h0q = []
            w1_first = []
            for q in range(4):  # block0 in 2-slab quarters
                t_ = ht_pool.tile([P, 2 * BLOCKS[0]], mm_dt, tag=f"ht0_{q}")
                nc.sync.dma_start(
                    t_[:],
                    htp[:, q * 2 * BLOCKS[0]:(q + 1) * 2 * BLOCKS[0]],
                )
                h0q.append(t_)
                if q == 0:  # the first m-pass's weights ride 2nd in line
                    t_ = wpool.tile([P, IN], mm_dt, tag="w1_0")
                    nc.sync.dma_start(t_[:], w1m[0:P, :])
                    w1_first.append(t_)

            def ht_slab(b, k):
                if b == 0:
                    return h0q[k // 2][:, (k % 2) * BLOCKS[0]:
                                      (k % 2 + 1) * BLOCKS[0]]
                t_ = ht_tiles[b]
                return t_[:, k * BLOCKS[b]:(k + 1) * BLOCKS[b]]

            w1_sb = list(w1_first)
            for m in range(1, MC1):
                t_ = wpool.tile([P, IN], mm_dt, tag=f"w1_{m}")
                nc.sync.dma_start(t_[:], w1m[m * P:(m + 1) * P, :])
                w1_sb.append(t_)
                if m == 1:  # first gelu needs the bias ~10us after mm0
                    b1_sb = cpool.tile([P, MC1], F32, tag="b1")
                    nc.sync.dma_start(b1_sb[:], b1t[:])
            t2 = ht_pool.tile([P, KC1 * BLOCKS[1]], mm_dt, tag="ht_1")
            nc.sync.dma_start(t2[:], htp[:, TOFF[1] * KC1:TOFF[2] * KC1])
            ht_tiles[1] = t2
            w2_sb = []
            for hc in range(MC1):
                t_ = wpool.tile([P, D], mm_dt, tag=f"w2_{hc}")
                nc.sync.dma_start(t_[:], w2[hc * P:(hc + 1) * P, :])
                w2_sb.append(t_)
            for b in range(2, NBLK):
                t_ = ht_pool.tile([P, KC1 * BLOCKS[b]], mm_dt, tag=f"ht_{b}")
                nc.sync.dma_start(
                    t_[:], htp[:, TOFF[b] * KC1:TOFF[b + 1] * KC1]
                )
                ht_tiles[b] = t_

            for b in range(NBLK):
                blk = BLOCKS[b]
                nbank = blk // 512
                # --- L1: hidT[m] = gelu((W1 block m).T @ hT + b1[m]) ---
                hid_sb = []
                for m in range(MC1):
                    banks = [
                        ps_pool.tile([P, 512], F32, tag="ps", name=f"ps{j}")
                        for j in range(nbank)
                    ]
                    for k in range(KC1):
                        for j in range(nbank):
                            nc.tensor.matmul(
                                banks[j][:],
                                w1_sb[m][:, k * P:(k + 1) * P],
                                ht_slab(b, k)[:, j * 512:(j + 1) * 512],
                                start=(k == 0),
                                stop=(k == KC1 - 1),
                            )
                    hm = hid_pool.tile([P, MAXBLK], mm_dt, tag="hid")
                    for j in range(nbank):
                        nc.scalar.activation(
                            hm[:, j * 512:(j + 1) * 512],
                            banks[j][:],
                            mybir.ActivationFunctionType.Gelu,
                            bias=b1_sb[:, m:m + 1],
                            scale=1.0,
                        )
                    hid_sb.append(hm)

                # --- L2 + chunked ReduceScatter per the schedule ---
                chunks = [c for c in CHUNKS if c[0] == b]
                ci = 0
                fe_chunk = None
                for dc in range(DC2):
                    blk_, dc_lo, dc_n = chunks[ci]
                    if dc == dc_lo:
                        fe_chunk = dram_pool.tile(
                            [dc_n * P, blk], rs_dt, tag=f"fe{dc_n}_{blk}",
                            name="fe_chunk", bufs=2,
                        )
                    banks = [
                        ps_pool.tile([P, 512], F32, tag="ps", name=f"ps{j}")
                        for j in range(nbank)
                    ]
                    for hc in range(MC1):
                        for j in range(nbank):
                            nc.tensor.matmul(
                                banks[j][:],
                                w2_sb[hc][:, dc * P:(dc + 1) * P],
                                hid_sb[hc][:, j * 512:(j + 1) * 512],
                                start=(hc == 0),
                                stop=(hc == MC1 - 1),
                            )
                    fe_sb = fe_pool.tile([P, MAXBLK], rs_dt, tag="fe")
                    for j in range(nbank):
                        nc.scalar.activation(
                            fe_sb[:, j * 512:(j + 1) * 512],
                            banks[j][:],
                            mybir.ActivationFunctionType.Copy,
                        )
                    nc.scalar.dma_start(
                        fe_chunk[(dc - dc_lo) * P:(dc - dc_lo + 1) * P, :],
                        fe_sb[:, 0:blk],
                    )
                    if dc == dc_lo + dc_n - 1:
                        rs_chunk = dram_pool.tile(
                            [dc_n * P // NCORES, blk], rs_dt,
                            tag=f"rs{dc_n}_{blk}", name="rs_chunk", bufs=2,
                        )
                        nc.gpsimd.collective_compute(
                            "ReduceScatter",
                            mybir.AluOpType.add,
                            replica_groups=[list(range(NCORES))],
                            ins=[fe_chunk[:]],
                            outs=[rs_chunk[:]],
                        )
                        off = out_off[(b, dc_lo)]
                        nc.sync.dma_start(
                            outs[blk][off:off + dc_n * P // NCORES, :],
                            rs_chunk[:],
                        )
                        ci += 1

    nc.finalize()
    return nc


def _get_nc(mm_dtype_name, rs_dtype_name):
    key = (mm_dtype_name, rs_dtype_name)
    if key not in _CACHE:
        _CACHE[key] = build(mm_dtype_name, rs_dtype_name)
    return _CACHE[key]


def _run(inputs, mm_dtype_name="float16", trace=False):
    from concourse.bass_utils import run_bass_kernel_spmd

    import ml_dtypes

    rs_dtype_name = os.environ.get("MOE_RS_DTYPE", "float16")
    np_mm = {"bfloat16": ml_dtypes.bfloat16, "float16": np.float16}[
        mm_dtype_name
    ]
    h = np.ascontiguousarray(np.asarray(inputs["h"], dtype=np.float32))
    hT = h.T.astype(np_mm)  # [IN, B]
    htp = np.concatenate(
        [
            np.ascontiguousarray(
                hT[:, TOFF[b]:TOFF[b + 1]]
                .reshape(KC1, P, BLOCKS[b]).transpose(1, 0, 2)
                .reshape(P, KC1 * BLOCKS[b])
            )
            for b in range(NBLK)
        ],
        axis=1,
    )
    gate_logits = np.asarray(inputs["gate_logits"], dtype=np.float64)
    W1 = np.asarray(inputs["W1"], dtype=np.float32)
    b1 = np.asarray(inputs["b1"], dtype=np.float32)
    W2 = np.asarray(inputs["W2"], dtype=np.float32)
    b2 = np.asarray(inputs["b2"], dtype=np.float32)

    # gate: softmax over E (uniform for zero logits); fold into W2 per expert
    z = np.exp(gate_logits - gate_logits.max())
    probs = (z / z.sum()).astype(np.float32)

    in_maps = []
    for e in range(NCORES):
        w1e = W1[e].astype(np_mm)                      # [IN, H]
        w1m = np.ascontiguousarray(
            w1e.reshape(KC1, P, MC1, P).transpose(2, 1, 0, 3)
            .reshape(MC1 * P, IN)
        )
        in_maps.append({
            "htp": htp,
            "w1m": w1m,
            "b1t": np.ascontiguousarray(b1[e].reshape(MC1, P).T),  # [P,MC1]
            "w2": np.ascontiguousarray((W2[e] * probs[e]).astype(np_mm)),
        })

    nc = _get_nc(mm_dtype_name, rs_dtype_name)
    res = run_bass_kernel_spmd(nc, in_maps, list(range(NCORES)), trace=trace)

    # Reassemble. Chunk (b, dc_lo, n): core r's rows in out<cols> are global
    # D rows dc_lo*128 + r*(n*16) + i, tokens TOFF[b]..TOFF[b+1].
    out_pos = {}
    feT = np.empty((D, B), dtype=np.float32)
    for r in range(NCORES):
        o_by_cols = {}
        pos = {}
        for (b, dc_lo, n) in CHUNKS:
            cols = BLOCKS[b]
            if cols not in o_by_cols:
                o_by_cols[cols] = np.asarray(
                    res.results[r][f"out{cols}"], dtype=np.float32
                )
                pos[cols] = 0
            rpc = n * P // NCORES
            o = o_by_cols[cols]
            feT[dc_lo * P + r * rpc: dc_lo * P + (r + 1) * rpc,
                TOFF[b]:TOFF[b + 1]] = o[pos[cols]:pos[cols] + rpc, :]
            pos[cols] += rpc
    final = feT.T.copy()
    final += (probs @ b2)[None, :]  # token-independent bias term
    return final, res


def kernel(**inputs):
    mm_dtype_name = os.environ.get("MOE_MM_DTYPE", "float16")
    final, _ = _run(inputs, mm_dtype_name=mm_dtype_name, trace=False)
    return final


# revision 19
# speedup vs baseline: 1.3308x; 1.2816x over previous
# Dense-MoE (all experts active) Trainium2 kernel, DATA-parallel over 8
# NeuronCores: core r owns tokens [r*512, (r+1)*512) and computes the full
# expert sum for them:
#   out_r = sum_e gelu(h_r @ W1[e] + b1[e]) @ (probs[e] * W2[e])
# The host unshard is a pure concatenation (plus the token-independent
# sum_e probs[e]*b2[e] bias term). No collectives.
#
# Why data-parallel: a NEFF that contains ANY collective runs the PE array
# at ~263ns per 512-col fp16 matmul pair (measured); a collective-free NEFF
# runs the identical stream at 216ns (full 2.4GHz) — a 22% static clock tax
# on the whole kernel. Both shardings need the same 2048 matmuls/core, so
# dropping the ReduceScatter is worth ~110us. Heavy concurrent DMA (the
# ~145GB/s weight streaming this design needs) does NOT affect the clock
# (measured).
#
# Layout: activations stay transposed on-chip; out is [D, tok].
#   htT  [IN, 512]  fp16 slabs, one per k-chunk, host pre-packed
#   hidT [H, 512]   = (W1[e] block).T @ htT per 128-row chunk, gelu+b1 (ACT)
#   acc  [D, 512]   fp32 SBUF accumulator over experts; L2 psum drains are
#                   added in by the DVE per Dc chunk
# All 8 experts' weights stream from DRAM through rolling slab pools
# (64MB/core over ~440us; pool-slot recycling provides the flow control).
import os
import sys

sys.path.insert(0, "/opt/trn_rl_repo")

import numpy as np

import concourse.mybir as mybir
from concourse import bacc, tile

B, E, IN, H, D = 4096, 8, 1024, 2048, 1024
NCORES = 8
P = 128
TOK = B // NCORES         # 512 tokens per core
KC1 = IN // P             # 8 contraction chunks, layer 1
MC1 = H // P              # 16 H chunks (layer-1 output rows)
DC2 = D // P              # 8 D chunks (layer-2 output rows)

F32 = mybir.dt.float32

_CACHE = {}


def build(mm_dtype_name="float16", act_name="Gelu"):
    mm_dt = getattr(mybir.dt, mm_dtype_name)
    assert mybir.dt.size(mm_dt) == 2, "matmul path requires a 16-bit dtype"
    nc = bacc.Bacc("TRN2", target_bir_lowering=False)

    # htp[p, k*TOK + t] = h[r*TOK + t, k*P + p] for this core's shard
    htp = nc.declare_dram_parameter("htp", [P, KC1 * TOK], mm_dt,
                                    isOutput=False)
    # w1m[(e*MC1 + m)*P + p, k*P + c] = W1[e][k*P + p, m*P + c]
    w1m = nc.declare_dram_parameter("w1m", [E * MC1 * P, IN], mm_dt,
                                    isOutput=False)
    # b1t[p, e*MC1 + m] = b1[e][m*P + p]
    b1t = nc.declare_dram_parameter("b1t", [P, E * MC1], F32, isOutput=False)
    # w2s[(e*MC1 + hc)*P + p, :] = probs[e] * W2[e][hc*P + p, :]
    w2s = nc.declare_dram_parameter("w2s", [E * MC1 * P, D], mm_dt,
                                    isOutput=False)
    out = nc.declare_dram_parameter("out", [D, TOK], F32, isOutput=True)

    with tile.TileContext(nc) as tc:
        with (
            tc.tile_pool(name="consts", bufs=1) as cpool,
            tc.tile_pool(name="w1p", bufs=24) as w1_pool,
            tc.tile_pool(name="w2p", bufs=40) as w2_pool,
            tc.tile_pool(name="hid", bufs=2 * MC1) as hid_pool,
            tc.tile_pool(name="acc", bufs=1) as acc_pool,
            tc.tile_pool(name="ps", bufs=8, space="PSUM") as ps_pool,
        ):
            ht_sb = cpool.tile([P, KC1 * TOK], mm_dt, tag="ht")
            nc.sync.dma_start(ht_sb[:], htp[:])
            b1_sb = cpool.tile([P, E * MC1], F32, tag="b1")
            nc.sync.dma_start(b1_sb[:], b1t[:])

            # weight slab DMAs are issued in consumption order on the sync
            # queue; the rolling pools stall the queue head until the slot's
            # previous consumer is done, which paces the ~145GB/s stream.
            w1_sb = {}
            w2_sb = {}

            def load_w1(e, m):
                t_ = w1_pool.tile([P, IN], mm_dt, tag="w1", name="w1s")
                nc.sync.dma_start(
                    t_[:], w1m[(e * MC1 + m) * P:(e * MC1 + m + 1) * P, :]
                )
                w1_sb[(e, m)] = t_

            def load_w2(e, hc):
                t_ = w2_pool.tile([P, D], mm_dt, tag="w2", name="w2s")
                nc.sync.dma_start(
                    t_[:], w2s[(e * MC1 + hc) * P:(e * MC1 + hc + 1) * P, :]
                )
                w2_sb[(e, hc)] = t_

            for m in range(MC1):
                load_w1(0, m)
            for hc in range(MC1):
                load_w2(0, hc)
            for m in range(MC1):
                load_w1(1, m)

            acc = [
                acc_pool.tile([P, TOK], F32, tag=f"acc{dc}", name=f"acc{dc}")
                for dc in range(DC2)
            ]

            for e in range(E):
                # issue the NEXT experts' weight DMAs; pool slots throttle
                # them to the right time
                if e + 1 < E:
                    for hc in range(MC1):
                        load_w2(e + 1, hc)
                if e + 2 < E:
                    for m in range(MC1):
                        load_w1(e + 2, m)

                # --- L1(e): hidT[m] = gelu((W1[e] blk m).T @ htT + b1) ---
                hid_sb = []
                for m in range(MC1):
                    bank = ps_pool.tile([P, TOK], F32, tag="ps", name="psb")
                    for k in range(KC1):
                        nc.tensor.matmul(
                            bank[:],
                            w1_sb[(e, m)][:, k * P:(k + 1) * P],
                            ht_sb[:, k * TOK:(k + 1) * TOK],
                            start=(k == 0),
                            stop=(k == KC1 - 1),
                        )
                    hm = hid_pool.tile([P, TOK], mm_dt, tag="hid")
                    nc.scalar.activation(
                        hm[:],
                        bank[:],
                        getattr(mybir.ActivationFunctionType, act_name),
                        bias=(0.0 if act_name == "Copy" else
                              b1_sb[:, e * MC1 + m:e * MC1 + m + 1]),
                        scale=1.0,
                    )
                    hid_sb.append(hm)
                    del w1_sb[(e, m)]

                # --- L2(e): acc[dc] (+)= (W2'[e] blk).T @ hidT ---
                for dc in range(DC2):
                    bank = ps_pool.tile([P, TOK], F32, tag="ps", name="psb")
                    for hc in range(MC1):
                        nc.tensor.matmul(
                            bank[:],
                            w2_sb[(e, hc)][:, dc * P:(dc + 1) * P],
                            hid_sb[hc][:],
                            start=(hc == 0),
                            stop=(hc == MC1 - 1),
                        )
                    if e == 0:
                        nc.vector.tensor_copy(acc[dc][:], bank[:])
                    else:
                        nc.vector.tensor_add(acc[dc][:], acc[dc][:], bank[:])
                    if e == E - 1:
                        nc.gpsimd.dma_start(
                            out[dc * P:(dc + 1) * P, :], acc[dc][:]
                        )
                for hc in range(MC1):
                    del w2_sb[(e, hc)]

    nc.finalize()
    return nc


def _get_nc(mm_dtype_name):
    if mm_dtype_name not in _CACHE:
        _CACHE[mm_dtype_name] = build(mm_dtype_name)
    return _CACHE[mm_dtype_name]


def _run(inputs, mm_dtype_name="float16", trace=False):
    from concourse.bass_utils import run_bass_kernel_spmd

    import ml_dtypes

    np_mm = {"bfloat16": ml_dtypes.bfloat16, "float16": np.float16}[
        mm_dtype_name
    ]
    h = np.ascontiguousarray(np.asarray(inputs["h"], dtype=np.float32))
    hT = h.T.astype(np_mm)  # [IN, B]
    gate_logits = np.asarray(inputs["gate_logits"], dtype=np.float64)
    W1 = np.asarray(inputs["W1"], dtype=np.float32)
    b1 = np.asarray(inputs["b1"], dtype=np.float32)
    W2 = np.asarray(inputs["W2"], dtype=np.float32)
    b2 = np.asarray(inputs["b2"], dtype=np.float32)

    # gate: softmax over E (uniform for zero logits); fold into W2 per expert
    z = np.exp(gate_logits - gate_logits.max())
    probs = (z / z.sum()).astype(np.float32)

    # weights are identical on every core; only the token shard differs
    w1m = np.ascontiguousarray(
        W1.astype(np_mm).reshape(E, KC1, P, MC1, P)
        .transpose(0, 3, 2, 1, 4).reshape(E * MC1 * P, IN)
    )
    w2sc = np.ascontiguousarray(
        (W2 * probs[:, None, None]).astype(np_mm).reshape(E * MC1 * P, D)
    )
    b1tt = np.ascontiguousarray(
        b1.reshape(E, MC1, P).transpose(2, 0, 1).reshape(P, E * MC1)
    )

    in_maps = []
    for r in range(NCORES):
        shard = hT[:, r * TOK:(r + 1) * TOK]          # [IN, TOK]
        htp = np.ascontiguousarray(
            shard.reshape(KC1, P, TOK).transpose(1, 0, 2)
            .reshape(P, KC1 * TOK)
        )
        in_maps.append({
            "htp": htp, "w1m": w1m, "b1t": b1tt, "w2s": w2sc,
        })

    nc = _get_nc(mm_dtype_name)
    res = run_bass_kernel_spmd(nc, in_maps, list(range(NCORES)), trace=trace)

    final = np.empty((B, D), dtype=np.float32)
    for r in range(NCORES):
        o = np.asarray(res.results[r]["out"], dtype=np.float32)  # [D, TOK]
        final[r * TOK:(r + 1) * TOK, :] = o.T
    final += (probs @ b2)[None, :]  # token-independent bias term
    return final, res


def kernel(**inputs):
    mm_dtype_name = os.environ.get("MOE_MM_DTYPE", "float16")
    final, _ = _run(inputs, mm_dtype_name=mm_dtype_name, trace=False)
    return final
